# revision 1
# baseline (speedup 1.0000x reference)
"""Trainium2 Bass kernel for nn_FSMNSeleNetV3 (FSMN stack + channel maxpool + decoder).

Self-contained: hardcodes all shapes from the problem spec and only imports
numpy + the concourse stack from /opt/trn_rl_repo.

Sharding: pure data parallel over batch. Each of the 8 cores processes 4
batches x 4 channels = 16 independent sequences of T=2048 tokens.

Layout: activations live as [feature_dim, time] in SBUF (features on
partitions). The host pre-transposes x to [B, C, F, T] so the DMA loads are
plain 2D loads. 64-channel tensors (shrink/FSMN stream) pack the two T/2
halves of a sequence onto 128 partitions so every engine sees full-width
tiles.

FSMN conv: 11 taps + residual. 8 taps + the residual identity run on the PE
as diagonal-matrix matmuls accumulating in PSUM (the two halves use disjoint
64x64 quadrants of the array, so they run concurrently); the remaining 3 taps
run as fused scalar_tensor_tensor FMAs on the DVE, with the first one reading
the conv PSUM directly as its addend. All matmuls use float32r (full-rate
fp32 streaming on the PE).
"""

import sys

sys.path.insert(0, "/opt/trn_rl_repo")
from contextlib import ExitStack

import numpy as np

import concourse.bass as bass  # noqa: F401
import concourse.mybir as mybir
import concourse.tile as tile
from concourse import bacc
from concourse.bass_utils import run_bass_kernel_spmd

F32 = mybir.dt.float32
F32R = mybir.dt.float32r
BF16 = mybir.dt.bfloat16
AF = mybir.ActivationFunctionType
OP = mybir.AluOpType

NCORES = 8
B, T, C, F = 32, 2048, 4, 120
DL, DP, L, LO, RO, S = 128, 64, 5, 10, 1, 5
BPC = B // NCORES  # batches per core
SEQ = BPC * C  # sequences per core
H = T // 2  # half-sequence length (halves stacked on partitions)
HALO_L = LO - 1  # 9 left halo columns
HW = H + HALO_L + RO  # h buffer width: 1034
NW = T // 512  # 512-token matmul windows per sequence

# conv tap split: delta in [-9..+1]; PE handles -9..-2 (+ residual identity),
# DVE handles -1, 0, +1 (tap table cols 8, 9, 10)
PE_DELTAS = list(range(-9, -1))  # 8 taps on the PE
NSLOT = len(PE_DELTAS) + 1  # + identity slot (always present, used when l>0)


def _mm(nc, out, lhsT, rhs, **kw):
    nc.tensor.matmul(out, lhsT, rhs, **kw)


def build_nc():
    nc = bacc.Bacc("TRN2", target_bir_lowering=False, debug=False, num_devices=NCORES)

    xt_d = nc.dram_tensor("xt", [SEQ, F, T], F32R, kind="ExternalInput")
    we0_d = nc.dram_tensor("we0", [F, DL], F32R, kind="ExternalInput")
    wedup_d = nc.dram_tensor("wedup", [L, 2 * DP, DL], F32R, kind="ExternalInput")
    ws_d = nc.dram_tensor("ws", [L, DL, DP], F32R, kind="ExternalInput")
    wd_d = nc.dram_tensor("wd", [DL, S], F32R, kind="ExternalInput")
    biases_d = nc.dram_tensor("biases", [DL, L + 1], F32, kind="ExternalInput")
    taps_d = nc.dram_tensor("taps", [2 * DP, L * 11], F32, kind="ExternalInput")
    diag_d = nc.dram_tensor("diag", [2 * DP, L * len(PE_DELTAS) * 2 * DP], BF16, kind="ExternalInput")
    ident_d = nc.dram_tensor("ident", [2 * DP, 2 * DP], F32R, kind="ExternalInput")
    bd_d = nc.dram_tensor("bd", [S, 1], F32, kind="ExternalInput")
    out_d = nc.dram_tensor("out", [BPC, S, T], F32, kind="ExternalOutput")

    with tile.TileContext(nc) as tc, ExitStack() as ctx:
        wp = ctx.enter_context(tc.tile_pool(name="weights", bufs=1))
        xp = ctx.enter_context(tc.tile_pool(name="x", bufs=3))
        ep = ctx.enter_context(tc.tile_pool(name="e", bufs=4))
        hp = ctx.enter_context(tc.tile_pool(name="h", bufs=4))
        op_ = ctx.enter_context(tc.tile_pool(name="o", bufs=4))
        fp = ctx.enter_context(tc.tile_pool(name="f", bufs=5))
        pp = ctx.enter_context(tc.tile_pool(name="pooled", bufs=2))
        osb = ctx.enter_context(tc.tile_pool(name="osb", bufs=2))
        ps = ctx.enter_context(tc.tile_pool(name="ps", bufs=4, space="PSUM"))
        psh = ctx.enter_context(tc.tile_pool(name="psh", bufs=4, space="PSUM"))

        # --- weights / constants (loaded once) ---
        we0_sb = wp.tile([F, DL], F32R)
        nc.sync.dma_start(out=we0_sb[:], in_=we0_d[:])
        wedup_sb = wp.tile([2 * DP, L * DL], F32R)
        ws_sb = wp.tile([DL, L * DP], F32R)
        for l in range(L):
            nc.sync.dma_start(out=wedup_sb[:, l * DL : (l + 1) * DL], in_=wedup_d[l])
            nc.sync.dma_start(out=ws_sb[:, l * DP : (l + 1) * DP], in_=ws_d[l])
        wd_sb = wp.tile([DL, S], F32R)
        nc.sync.dma_start(out=wd_sb[:], in_=wd_d[:])
        bias_sb = wp.tile([DL, L + 1], F32)
        nc.sync.dma_start(out=bias_sb[:], in_=biases_d[:])
        taps_sb = wp.tile([2 * DP, L * 11], F32)
        nc.sync.dma_start(out=taps_sb[:], in_=taps_d[:])
        diag_sb = wp.tile([2 * DP, L * len(PE_DELTAS) * 2 * DP], BF16)
        nc.sync.dma_start(out=diag_sb[:], in_=diag_d[:])
        ident_sb = wp.tile([2 * DP, 2 * DP], F32R)
        nc.sync.dma_start(out=ident_sb[:], in_=ident_d[:])
        bd_sb = wp.tile([S, 1], F32)
        nc.sync.dma_start(out=bd_sb[:], in_=bd_d[:])
        zero_sb = wp.tile([2 * DP, HALO_L], F32)
        nc.gpsimd.memset(zero_sb[:], 0.0)

        def tap(l, j):
            return taps_sb[:, l * 11 + j : l * 11 + j + 1]

        def diag(l, s):
            col = (l * len(PE_DELTAS) + s) * 2 * DP
            return diag_sb[:, col : col + 2 * DP]

        for b in range(BPC):
            f_tiles = []
            for c in range(C):
                seq = b * C + c

                x_sb = xp.tile([F, T], F32R)
                nc.sync.dma_start(out=x_sb[:], in_=xt_d[seq])

                # ---- unit-0 expand: relu(x @ We0 + be0), K=120 ----
                e_sb = ep.tile([DL, T], F32R)
                for w in range(NW):
                    pe = ps.tile([DL, 512], F32, tag="ps")
                    _mm(nc, pe[:], we0_sb[:], x_sb[:, w * 512 : (w + 1) * 512])
                    nc.scalar.activation(
                        e_sb[:, w * 512 : (w + 1) * 512],
                        pe[:],
                        AF.Relu,
                        bias=bias_sb[:, 0:1],
                        scale=1.0,
                    )

                o_prev = None
                for l in range(L):
                    if l > 0:
                        # ---- expand l: relu(o @ We[l-1] + be[l]), K=64,
                        # halves row-tiled concurrently ----
                        e_sb = ep.tile([DL, T], F32R)
                        for half in range(2):
                            q = half * DP
                            lhsT = wedup_sb[q : q + DP, (l - 1) * DL : l * DL]
                            for w in range(2):
                                pe = ps.tile([DL, 512], F32, tag="ps")
                                _mm(
                                    nc,
                                    pe[:],
                                    lhsT,
                                    o_prev[q : q + DP, w * 512 : (w + 1) * 512],
                                    tile_position=(q, 0),
                                )
                                col = (half * 2 + w) * 512
                                nc.scalar.activation(
                                    e_sb[:, col : col + 512],
                                    pe[:],
                                    AF.Relu,
                                    bias=bias_sb[:, l : l + 1],
                                    scale=1.0,
                                )

                    # ---- shrink l: h = e @ Ws[l], halves stacked into one
                    # PSUM bank via col tiling ----
                    h_sb = hp.tile([2 * DP, HW], BF16)
                    ws_l = ws_sb[:, l * DP : (l + 1) * DP]
                    phs = []
                    for w in range(2):
                        dst = slice(HALO_L + w * 512, HALO_L + (w + 1) * 512)
                        pha = psh.tile([DP, 512], F32, tag="ph")
                        phb = psh.tile([DP, 512], F32, tag="ph")
                        _mm(nc, pha[:], ws_l, e_sb[:, w * 512 : (w + 1) * 512])
                        _mm(nc, phb[:], ws_l, e_sb[:, H + w * 512 : H + (w + 1) * 512])
                        nc.vector.tensor_copy(h_sb[0:DP, dst], pha[:])
                        nc.scalar.copy(h_sb[DP : 2 * DP, dst], phb[:])
                        phs.append((pha, phb))

                    # ---- halo columns ----
                    nc.vector.tensor_copy(h_sb[0:DP, 0:HALO_L], zero_sb[0:DP, :])
                    nc.vector.tensor_copy(h_sb[DP : 2 * DP, H + HALO_L : HW], zero_sb[DP : 2 * DP, 0:RO])
                    nc.vector.tensor_copy(
                        h_sb[DP : 2 * DP, 0:HALO_L], h_sb[0:DP, H : H + HALO_L]
                    )
                    nc.vector.tensor_copy(
                        h_sb[0:DP, H + HALO_L : HW],
                        h_sb[DP : 2 * DP, HALO_L : HALO_L + 1],
                    )

                    # ---- FSMN conv ----
                    # PE part: 8 far-left taps (+ residual identity if l>0) as
                    # diagonal matmuls; halves in disjoint 64x64 quadrants.
                    pcs = []
                    for w in range(2):
                        pc = ps.tile([2 * DP, 512], F32, tag="ps")
                        nmm = len(PE_DELTAS) + (1 if l > 0 else 0)
                        i = 0
                        if l > 0:
                            _mm(
                                nc,
                                pc[:],
                                ident_sb[:],
                                o_prev[:, w * 512 : (w + 1) * 512],
                                start=True,
                                stop=(nmm == 1),
                            )
                            i = 1
                        for s, d in enumerate(PE_DELTAS):
                            a = HALO_L + d + w * 512
                            _mm(
                                nc,
                                pc[:],
                                diag(l, s),
                                h_sb[:, a : a + 512],
                                start=(i == 0),
                                stop=(i == nmm - 1),
                            )
                            i += 1
                        pcs.append(pc)

                    # DVE part (one PSUM operand per op): seed with
                    # delta=-1 adding the PE conv PSUM, then delta=0 from the
                    # exact fp32 shrink PSUM, then delta=+1 from bf16 h.
                    o_new = op_.tile([2 * DP, H], F32R)
                    for w in range(2):
                        ws_ = slice(w * 512, (w + 1) * 512)
                        a = HALO_L - 1 + w * 512
                        nc.vector.scalar_tensor_tensor(
                            o_new[:, ws_],
                            h_sb[:, a : a + 512],
                            tap(l, 8),
                            pcs[w][:],
                            OP.mult,
                            OP.add,
                        )
                    for w in range(2):
                        ws_ = slice(w * 512, (w + 1) * 512)
                        pha, phb = phs[w]
                        t9 = tap(l, 9)
                        nc.vector.scalar_tensor_tensor(
                            o_new[0:DP, ws_],
                            pha[:],
                            t9[0:DP, :],
                            o_new[0:DP, ws_],
                            OP.mult,
                            OP.add,
                        )
                        nc.vector.scalar_tensor_tensor(
                            o_new[DP : 2 * DP, ws_],
                            phb[:],
                            t9[DP : 2 * DP, :],
                            o_new[DP : 2 * DP, ws_],
                            OP.mult,
                            OP.add,
                        )
                    nc.vector.scalar_tensor_tensor(
                        o_new[:],
                        h_sb[:, HALO_L + 1 : HALO_L + 1 + H],
                        tap(l, 10),
                        o_new[:],
                        OP.mult,
                        OP.add,
                    )
                    o_prev = o_new

                # ---- final expand: relu(o @ We2 + be2) ----
                f_sb = fp.tile([DL, T], F32R)
                for half in range(2):
                    q = half * DP
                    lhsT = wedup_sb[q : q + DP, 4 * DL : 5 * DL]
                    for w in range(2):
                        pe = ps.tile([DL, 512], F32, tag="ps")
                        _mm(
                            nc,
                            pe[:],
                            lhsT,
                            o_prev[q : q + DP, w * 512 : (w + 1) * 512],
                            tile_position=(q, 0),
                        )
                        col = (half * 2 + w) * 512
                        nc.scalar.activation(
                            f_sb[:, col : col + 512],
                            pe[:],
                            AF.Relu,
                            bias=bias_sb[:, L : L + 1],
                            scale=1.0,
                        )
                f_tiles.append(f_sb)

            # ---- channel maxpool + decoder for this batch ----
            pooled = pp.tile([DL, T], F32R)
            nc.vector.tensor_max(pooled[:], f_tiles[0][:], f_tiles[1][:])
            nc.vector.tensor_max(pooled[:], pooled[:], f_tiles[2][:])
            nc.vector.tensor_max(pooled[:], pooled[:], f_tiles[3][:])

            out_sb = osb.tile([S, T], F32)
            for w in range(NW):
                pd = ps.tile([S, 512], F32, tag="ps")
                _mm(nc, pd[:], wd_sb[:], pooled[:, w * 512 : (w + 1) * 512])
                nc.scalar.activation(
                    out_sb[:, w * 512 : (w + 1) * 512],
                    pd[:],
                    AF.Identity,
                    bias=bd_sb[:, 0:1],
                    scale=1.0,
                )
            nc.sync.dma_start(out=out_d[b], in_=out_sb[:])

    nc.compile()
    return nc


_NC = None


def get_nc():
    global _NC
    if _NC is None:
        _NC = build_nc()
    return _NC


def prep_in_maps(x, We0, be0, Ws0, wl0, wr0, We, be, Ws, wl, wr, We2, be2, Wd, bd):
    xt = np.ascontiguousarray(x.transpose(0, 2, 3, 1), dtype=np.float32)  # [B,C,F,T]

    wedup = np.stack(
        [np.concatenate([w, w], axis=0) for w in [We[0], We[1], We[2], We[3], We2]]
    ).astype(np.float32)  # [L, 128, 128]
    ws_all = np.stack([Ws0, Ws[0], Ws[1], Ws[2], Ws[3]]).astype(np.float32)
    biases = np.stack([be0, be[0], be[1], be[2], be[3], be2], axis=1).astype(
        np.float32
    )  # [128, 6]

    wl_full = np.concatenate([wl0[None], wl], axis=0)  # [L, 10, 64]
    wr_full = np.concatenate([wr0[None], wr], axis=0)  # [L, 1, 64]
    taps64 = np.concatenate([wl_full, wr_full], axis=1).copy()  # [L, 11, 64]
    taps64[:, LO - 1, :] += 1.0  # conv identity term (o = h + left + right)
    taps = np.tile(
        taps64.transpose(2, 0, 1).reshape(DP, L * 11), (2, 1)
    )  # [128, 55], col = l*11 + j
    taps = np.ascontiguousarray(taps, dtype=np.float32)

    # diagonal tap matrices for the PE conv: slots 0..7 = taps delta=-9..-2
    # (tap table cols 0..7), slot 8 = identity (residual)
    import ml_dtypes

    npe = len(PE_DELTAS)
    diag = np.zeros((L, npe, 2 * DP, 2 * DP), np.float32)
    for l in range(L):
        for s in range(npe):
            np.fill_diagonal(diag[l, s], np.tile(taps64[l, s, :], 2))
    diag2 = diag.transpose(2, 0, 1, 3).reshape(2 * DP, L * npe * 2 * DP)
    diag2 = np.ascontiguousarray(diag2).astype(ml_dtypes.bfloat16)
    ident = np.eye(2 * DP, dtype=np.float32)

    shared = dict(
        we0=np.ascontiguousarray(We0, dtype=np.float32),
        wedup=wedup,
        ws=ws_all,
        wd=np.ascontiguousarray(Wd, dtype=np.float32),
        biases=np.ascontiguousarray(biases),
        taps=taps,
        diag=diag2,
        ident=ident,
        bd=np.ascontiguousarray(bd.reshape(S, 1), dtype=np.float32),
    )
    in_maps = []
    for k in range(NCORES):
        xs = xt[k * BPC : (k + 1) * BPC].reshape(SEQ, F, T)
        m = dict(shared)
        m["xt"] = np.ascontiguousarray(xs)
        in_maps.append(m)
    return in_maps


def postprocess(results):
    full = np.concatenate([r["out"] for r in results], axis=0)  # [B, S, T]
    return np.ascontiguousarray(full.transpose(0, 2, 1))  # [B, T, S]


def kernel(**inputs):
    nc = get_nc()
    in_maps = prep_in_maps(**inputs)
    res = run_bass_kernel_spmd(nc, in_maps, core_ids=list(range(NCORES)))
    return postprocess(res.results)



# revision 4
# speedup vs baseline: 1.4788x; 1.4788x over previous
"""Trainium2 Bass kernel for nn_FSMNSeleNetV3 (FSMN stack + channel maxpool + decoder).

Self-contained: hardcodes all shapes from the problem spec and only imports
numpy + the concourse stack from /opt/trn_rl_repo.

Sharding: pure data parallel over batch. Each of the 8 cores processes 4
batches x 4 channels = 16 independent sequences of T=2048 tokens.

Layout: activations use an even/odd time-split layout. The 128-dim expand
stream e is [128 feat, T] with columns 0:1024 = even times, 1024:2048 = odd
times. The 64-dim shrink/FSMN stream uses an interleaved pair layout:
partition 2c holds channel c at even times, partition 2c+1 at odd times,
giving [128, 1024] tiles with full partition utilization.

FSMN conv: in pair layout the 11-tap depthwise conv collapses to 7
pair-shift matmuls per 512-column window. Each pass is a full 128x128 bf16
matmul whose weight is a banded matrix of per-channel 2x2 blocks (built on
the host); all taps accumulate in fp32 PSUM. The conv identity term
(o = h + left + right) is folded into the k=0 tap weights; the layer
residual is folded into the PSUM evacuation as a DVE tensor_tensor add.

The interleaved layout keeps every matmul destination at PSUM base
partition 0 (the ISA rejects dst base 64): shrink even/odd are two M=128
matmuls with zero-interleaved weight columns accumulating into the same
bank; expand even/odd are two K=128 matmuls with zero-interleaved weight
rows. PSUM evacuations are single wide-FD instructions to amortize the
fixed per-op engine overheads.
"""

import sys

sys.path.insert(0, "/opt/trn_rl_repo")
from contextlib import ExitStack

import numpy as np

import concourse.bass as bass  # noqa: F401
import concourse.mybir as mybir
import concourse.tile as tile
from concourse import bacc
from concourse.bass_utils import run_bass_kernel_spmd

F32 = mybir.dt.float32
F32R = mybir.dt.float32r
BF16 = mybir.dt.bfloat16
AF = mybir.ActivationFunctionType
OP = mybir.AluOpType

NCORES = 8
B, T, C, F = 32, 2048, 4, 120
DL, DP, L, LO, RO, S = 128, 64, 5, 10, 1, 5
BPC = B // NCORES  # batches per core
SEQ = BPC * C  # sequences per core
NP_ = T // 2  # pair columns per sequence (1024)
HALO = 5  # left pair halo (k down to -5)
HW_ = HALO + NP_ + 1  # h buffer width: 1030
NK = 7  # pair-shift passes, k = kk - 5 in [-5 .. +1]
NH = 4  # static h buffers


def build_nc():
    nc = bacc.Bacc("TRN2", target_bir_lowering=False, debug=False, num_devices=NCORES)

    xt_d = nc.dram_tensor("xt", [SEQ, F, T], F32R, kind="ExternalInput")
    we0_d = nc.dram_tensor("we0", [F, DL], F32R, kind="ExternalInput")
    weil_d = nc.dram_tensor("weil", [L, 2, DL, DL], F32R, kind="ExternalInput")
    wsil_d = nc.dram_tensor("wsil", [L, 2, DL, DL], F32R, kind="ExternalInput")
    wconv_d = nc.dram_tensor("wconv", [2 * DP, L * NK * 2 * DP], BF16, kind="ExternalInput")
    wd_d = nc.dram_tensor("wd", [DL, S], BF16, kind="ExternalInput")
    biases_d = nc.dram_tensor("biases", [DL, L + 1], F32, kind="ExternalInput")
    bd_d = nc.dram_tensor("bd", [S, 1], F32, kind="ExternalInput")
    out_d = nc.dram_tensor("out", [BPC, S, T], F32, kind="ExternalOutput")

    with tile.TileContext(nc) as tc, ExitStack() as ctx:
        wp = ctx.enter_context(tc.tile_pool(name="weights", bufs=1))
        xp = ctx.enter_context(tc.tile_pool(name="x", bufs=3))
        ep = ctx.enter_context(tc.tile_pool(name="e", bufs=2))
        op_ = ctx.enter_context(tc.tile_pool(name="o", bufs=3))
        fp = ctx.enter_context(tc.tile_pool(name="f", bufs=5))
        pp = ctx.enter_context(tc.tile_pool(name="pooled", bufs=2))
        osb = ctx.enter_context(tc.tile_pool(name="osb", bufs=2))
        eps = ctx.enter_context(tc.tile_pool(name="eps", bufs=3, space="PSUM"))
        hps = ctx.enter_context(tc.tile_pool(name="hps", bufs=1, space="PSUM"))
        cps = ctx.enter_context(tc.tile_pool(name="cps", bufs=2, space="PSUM"))
        dps = ctx.enter_context(tc.tile_pool(name="dps", bufs=1, space="PSUM"))

        # --- weights / constants (loaded once) ---
        we0_sb = wp.tile([F, DL], F32R)
        nc.sync.dma_start(out=we0_sb[:], in_=we0_d[:])
        weil_sb = wp.tile([DL, L * 2 * DL], F32R)
        wsil_sb = wp.tile([DL, L * 2 * DL], F32R)
        for l in range(L):
            for half in range(2):
                col = (l * 2 + half) * DL
                nc.sync.dma_start(out=weil_sb[:, col : col + DL], in_=weil_d[l, half])
                nc.sync.dma_start(out=wsil_sb[:, col : col + DL], in_=wsil_d[l, half])
        wconv_sb = wp.tile([2 * DP, L * NK * 2 * DP], BF16)
        nc.sync.dma_start(out=wconv_sb[:], in_=wconv_d[:])
        wd_sb = wp.tile([DL, S], BF16)
        nc.sync.dma_start(out=wd_sb[:], in_=wd_d[:])
        bias_sb = wp.tile([DL, L + 1], F32)
        nc.sync.dma_start(out=bias_sb[:], in_=biases_d[:])
        bd_sb = wp.tile([S, 1], F32)
        nc.sync.dma_start(out=bd_sb[:], in_=bd_d[:])

        # static h buffers: halo columns zeroed once, data region rewritten
        # per (seq, layer) via the shrink evacuation
        h_tiles = []
        for i in range(NH):
            t = wp.tile([2 * DP, HW_], BF16, tag=f"h{i}", name=f"h{i}")
            nc.gpsimd.memset(t[:, 0:HALO], 0.0)
            nc.gpsimd.memset(t[:, HALO + NP_ : HW_], 0.0)
            h_tiles.append(t)

        def expand(dst_sb, lcol, bias_col, o_prev):
            # dst[:, half*NP_ + t] = relu(We^T o_prev_half[:, t] + bias)
            # o_prev interleaved; weight rows zero-interleaved per half.
            for half in range(2):
                wcol = (lcol * 2 + half) * DL
                for w in range(2):
                    pe = eps.tile([DL, 512], F32, tag="pe", name="pe")
                    nc.tensor.matmul(
                        pe[:],
                        weil_sb[:, wcol : wcol + DL],
                        o_prev[:, w * 512 : (w + 1) * 512],
                    )
                    nc.scalar.activation(
                        dst_sb[:, half * NP_ + w * 512 : half * NP_ + (w + 1) * 512],
                        pe[:],
                        AF.Relu,
                        bias=bias_sb[:, bias_col : bias_col + 1],
                        scale=1.0,
                    )

        hctr = 0
        for b in range(BPC):
            f_tiles = []
            for c in range(C):
                seq = b * C + c

                x_sb = xp.tile([F, T], F32R)
                nc.sync.dma_start(out=x_sb[:], in_=xt_d[seq])

                # ---- unit-0 expand: relu(x @ We0 + be0), K=120 ----
                e_sb = ep.tile([DL, T], F32R)
                for w in range(4):
                    pe = eps.tile([DL, 512], F32, tag="pe", name="pe")
                    nc.tensor.matmul(
                        pe[:], we0_sb[:], x_sb[:, w * 512 : (w + 1) * 512]
                    )
                    nc.scalar.activation(
                        e_sb[:, w * 512 : (w + 1) * 512],
                        pe[:],
                        AF.Relu,
                        bias=bias_sb[:, 0:1],
                        scale=1.0,
                    )

                o_prev = None
                for l in range(L):
                    if l > 0:
                        e_sb = ep.tile([DL, T], F32R)
                        expand(e_sb, l - 1, l, o_prev)

                    # ---- shrink l: ev/od accumulate into one bank with
                    # zero-interleaved weight columns ----
                    h_ps = hps.tile([2 * DP, NP_], F32)
                    for w in range(2):
                        for half in range(2):
                            wcol = (l * 2 + half) * DL
                            nc.tensor.matmul(
                                h_ps[:, w * 512 : (w + 1) * 512],
                                wsil_sb[:, wcol : wcol + DL],
                                e_sb[:, half * NP_ + w * 512 : half * NP_ + (w + 1) * 512],
                                start=(half == 0),
                                stop=(half == 1),
                            )
                    h_sb = h_tiles[hctr % NH]
                    hctr += 1
                    nc.vector.tensor_copy(h_sb[:, HALO : HALO + NP_], h_ps[:])

                    # ---- FSMN conv: 7 pair-shift passes per window ----
                    o_new = op_.tile([2 * DP, NP_], F32R)
                    for w in range(2):
                        cp = cps.tile([2 * DP, 512], F32, name="cp")
                        for kk in range(NK):
                            ck = (l * NK + kk) * 2 * DP
                            nc.tensor.matmul(
                                cp[:],
                                wconv_sb[:, ck : ck + 2 * DP],
                                h_sb[:, w * 512 + kk : w * 512 + kk + 512],
                                start=(kk == 0),
                                stop=(kk == NK - 1),
                            )
                        # evacuate conv PSUM (+ residual for l>0)
                        ws_ = slice(w * 512, (w + 1) * 512)
                        if l == 0:
                            nc.vector.tensor_copy(o_new[:, ws_], cp[:])
                        else:
                            nc.vector.tensor_tensor(
                                o_new[:, ws_], cp[:], o_prev[:, ws_], OP.add
                            )
                    o_prev = o_new

                # ---- final expand: relu(o @ We2 + be2) -> bf16 ----
                f_sb = fp.tile([DL, T], BF16)
                expand(f_sb, L - 1, L, o_prev)
                f_tiles.append(f_sb)

            # ---- channel maxpool + decoder for this batch ----
            pooled = pp.tile([DL, T], BF16)
            nc.vector.tensor_max(pooled[:], f_tiles[0][:], f_tiles[1][:])
            nc.vector.tensor_max(pooled[:], pooled[:], f_tiles[2][:])
            nc.vector.tensor_max(pooled[:], pooled[:], f_tiles[3][:])

            out_sb = osb.tile([S, T], F32)
            for w in range(T // 512):
                pd = dps.tile([S, 512], F32, name="pd")
                nc.tensor.matmul(pd[:], wd_sb[:], pooled[:, w * 512 : (w + 1) * 512])
                nc.scalar.activation(
                    out_sb[:, w * 512 : (w + 1) * 512],
                    pd[:],
                    AF.Identity,
                    bias=bd_sb[:, 0:1],
                    scale=1.0,
                )
            nc.sync.dma_start(out=out_d[b], in_=out_sb[:])

    nc.compile()
    return nc


_NC = None


def get_nc():
    global _NC
    if _NC is None:
        _NC = build_nc()
    return _NC


def prep_in_maps(x, We0, be0, Ws0, wl0, wr0, We, be, Ws, wl, wr, We2, be2, Wd, bd):
    import ml_dtypes

    # x [B, T, C, F] -> [B, C, F, T] with even|odd time blocks
    xb = np.ascontiguousarray(x.transpose(0, 2, 3, 1), dtype=np.float32)
    xt = np.concatenate([xb[..., 0::2], xb[..., 1::2]], axis=-1)  # [B, C, F, T]

    ev = 2 * np.arange(DP)
    od = ev + 1

    # expand weights, zero-interleaved rows: o interleaved -> e
    we_list = [We[0], We[1], We[2], We[3], We2]
    weil = np.zeros((L, 2, DL, DL), np.float32)
    for l in range(L):
        weil[l, 0][ev, :] = we_list[l]
        weil[l, 1][od, :] = we_list[l]

    # shrink weights, zero-interleaved columns: e -> h interleaved
    ws_list = [Ws0, Ws[0], Ws[1], Ws[2], Ws[3]]
    wsil = np.zeros((L, 2, DL, DL), np.float32)
    for l in range(L):
        wsil[l, 0][:, ev] = ws_list[l]
        wsil[l, 1][:, od] = ws_list[l]

    biases = np.stack([be0, be[0], be[1], be[2], be[3], be2], axis=1).astype(
        np.float32
    )  # [128, 6]

    wl_full = np.concatenate([wl0[None], wl], axis=0)  # [L, 10, 64]
    wr_full = np.concatenate([wr0[None], wr], axis=0)  # [L, 1, 64]
    taps64 = np.concatenate([wl_full, wr_full], axis=1).copy()  # [L, 11, 64], j = d+9
    taps64[:, LO - 1, :] += 1.0  # conv identity term (o = h + left + right)

    # pair-shift conv weights Wc[l, kk][p_in, p_out] in interleaved layout,
    # k = kk - 5: [2c,2c]=t_{2k}, [2c+1,2c+1]=t_{2k}, [2c+1,2c]=t_{2k+1},
    # [2c,2c+1]=t_{2k-1}
    Wc = np.zeros((L, NK, 2 * DP, 2 * DP), np.float32)
    for l in range(L):
        for kk in range(NK):
            k = kk - 5
            d = 2 * k
            if -9 <= d <= 1:
                Wc[l, kk][ev, ev] = taps64[l, d + 9]
                Wc[l, kk][od, od] = taps64[l, d + 9]
            d = 2 * k + 1
            if -9 <= d <= 1:
                Wc[l, kk][od, ev] = taps64[l, d + 9]
            d = 2 * k - 1
            if -9 <= d <= 1:
                Wc[l, kk][ev, od] = taps64[l, d + 9]
    wconv = np.ascontiguousarray(
        Wc.transpose(2, 0, 1, 3).reshape(2 * DP, L * NK * 2 * DP)
    ).astype(ml_dtypes.bfloat16)

    shared = dict(
        we0=np.ascontiguousarray(We0, dtype=np.float32),
        weil=weil,
        wsil=wsil,
        wconv=wconv,
        wd=np.ascontiguousarray(Wd, dtype=np.float32).astype(ml_dtypes.bfloat16),
        biases=np.ascontiguousarray(biases),
        bd=np.ascontiguousarray(bd.reshape(S, 1), dtype=np.float32),
    )
    in_maps = []
    for k in range(NCORES):
        xs = xt[k * BPC : (k + 1) * BPC].reshape(SEQ, F, T)
        m = dict(shared)
        m["xt"] = np.ascontiguousarray(xs)
        in_maps.append(m)
    return in_maps


def postprocess(results):
    full = np.concatenate([r["out"] for r in results], axis=0)  # [B, S, T] ev|od
    res = np.empty((B, T, S), np.float32)
    res[:, 0::2, :] = full[:, :, :NP_].transpose(0, 2, 1)
    res[:, 1::2, :] = full[:, :, NP_:].transpose(0, 2, 1)
    return res


def kernel(**inputs):
    nc = get_nc()
    in_maps = prep_in_maps(**inputs)
    res = run_bass_kernel_spmd(nc, in_maps, core_ids=list(range(NCORES)))
    return postprocess(res.results)


# revision 7
# speedup vs baseline: 2.1925x; 1.4826x over previous
"""Trainium2 Bass kernel for nn_FSMNSeleNetV3 (FSMN stack + channel maxpool + decoder).

Self-contained: hardcodes all shapes from the problem spec and only imports
numpy + the concourse stack from /opt/trn_rl_repo.

Sharding: pure data parallel over batch. Each of the 8 cores processes 4
batches x 4 channels = 16 independent sequences of T=2048 tokens.

Layout: activations use an even/odd time-split layout. The 128-dim expand
stream e is [128 feat, T] with columns 0:1024 = even times, 1024:2048 = odd
times. The 64-dim shrink/FSMN stream uses an interleaved pair layout:
partition 2c holds channel c at even times, partition 2c+1 at odd times,
giving [128, 1024] tiles with full partition utilization.

FSMN conv: in pair layout the 11-tap depthwise conv collapses to 7
pair-shift matmuls per 512-column window. Each pass is a full 128x128 bf16
matmul whose weight is a banded matrix of per-channel 2x2 blocks (built on
the host); all taps accumulate in fp32 PSUM. The conv identity term
(o = h + left + right) is folded into the k=0 tap weights; the layer
residual is folded into the PSUM evacuation as a DVE tensor_tensor add.

The interleaved layout keeps every matmul destination at PSUM base
partition 0 (the ISA rejects dst base 64): shrink even/odd are two M=128
matmuls with zero-interleaved weight columns accumulating into the same
bank; expand even/odd are two K=128 matmuls with zero-interleaved weight
rows.

Scheduling: sequences are emitted in software-pipelined pairs (stage-
interleaved) so the PE always has an independent matmul stream to fill
cross-engine latency gaps (keeps the PE HAM clock-gate warm). Matmuls that
share a stationary operand are emitted back-to-back (weight-major conv and
shrink loops); PSUM evacuations are single wide-FD instructions.
"""

import sys

sys.path.insert(0, "/opt/trn_rl_repo")
from contextlib import ExitStack

import numpy as np

import concourse.bass as bass  # noqa: F401
import concourse.mybir as mybir
import concourse.tile as tile
from concourse import bacc
from concourse.bass_utils import run_bass_kernel_spmd

F32 = mybir.dt.float32
F32R = mybir.dt.float32r
BF16 = mybir.dt.bfloat16
AF = mybir.ActivationFunctionType
OP = mybir.AluOpType

NCORES = 8
B, T, C, F = 32, 2048, 4, 120
DL, DP, L, LO, RO, S = 128, 64, 5, 10, 1, 5
BPC = B // NCORES  # batches per core
SEQ = BPC * C  # sequences per core
NP_ = T // 2  # pair columns per sequence (1024)
HALO = 5  # left pair halo (k down to -5)
HW_ = HALO + NP_ + 1  # h buffer width: 1030
NK = 7  # pair-shift passes, k = kk - 5 in [-5 .. +1]
NH = 6  # static h buffers


def build_nc():
    nc = bacc.Bacc("TRN2", target_bir_lowering=False, debug=False, num_devices=NCORES)

    xt_d = nc.dram_tensor("xt", [SEQ, F, T], F32R, kind="ExternalInput")
    we0_d = nc.dram_tensor("we0", [F, DL], F32R, kind="ExternalInput")
    weil_d = nc.dram_tensor("weil", [L, 2, DL, DL], F32R, kind="ExternalInput")
    wsil_d = nc.dram_tensor("wsil", [L, 2, DL, DL], F32R, kind="ExternalInput")
    wconv_d = nc.dram_tensor("wconv", [2 * DP, L * NK * 2 * DP], BF16, kind="ExternalInput")
    wd_d = nc.dram_tensor("wd", [DL, S], BF16, kind="ExternalInput")
    biases_d = nc.dram_tensor("biases", [DL, L + 1], F32, kind="ExternalInput")
    bd_d = nc.dram_tensor("bd", [S, 1], F32, kind="ExternalInput")
    out_d = nc.dram_tensor("out", [BPC, S, T], F32, kind="ExternalOutput")

    with tile.TileContext(nc) as tc, ExitStack() as ctx:
        wp = ctx.enter_context(tc.tile_pool(name="weights", bufs=1))
        xp = ctx.enter_context(tc.tile_pool(name="x", bufs=4))
        ep = ctx.enter_context(tc.tile_pool(name="e", bufs=3))
        op_ = ctx.enter_context(tc.tile_pool(name="o", bufs=4))
        fp = ctx.enter_context(tc.tile_pool(name="f", bufs=6))
        pp = ctx.enter_context(tc.tile_pool(name="pooled", bufs=2))
        osb = ctx.enter_context(tc.tile_pool(name="osb", bufs=2))
        eps = ctx.enter_context(tc.tile_pool(name="eps", bufs=3, space="PSUM"))
        hps = ctx.enter_context(tc.tile_pool(name="hps", bufs=2, space="PSUM"))
        cps = ctx.enter_context(tc.tile_pool(name="cps", bufs=3, space="PSUM"))

        # --- weights / constants (loaded once) ---
        we0_sb = wp.tile([F, DL], F32R)
        nc.sync.dma_start(out=we0_sb[:], in_=we0_d[:])
        weil_sb = wp.tile([DL, L * 2 * DL], F32R)
        wsil_sb = wp.tile([DL, L * 2 * DL], F32R)
        for l in range(L):
            for half in range(2):
                col = (l * 2 + half) * DL
                nc.sync.dma_start(out=weil_sb[:, col : col + DL], in_=weil_d[l, half])
                nc.sync.dma_start(out=wsil_sb[:, col : col + DL], in_=wsil_d[l, half])
        wconv_sb = wp.tile([2 * DP, L * NK * 2 * DP], BF16)
        nc.sync.dma_start(out=wconv_sb[:], in_=wconv_d[:])
        wd_sb = wp.tile([DL, S], BF16)
        nc.sync.dma_start(out=wd_sb[:], in_=wd_d[:])
        bias_sb = wp.tile([DL, L + 1], F32)
        nc.sync.dma_start(out=bias_sb[:], in_=biases_d[:])
        bd_sb = wp.tile([S, 1], F32)
        nc.sync.dma_start(out=bd_sb[:], in_=bd_d[:])

        # static h buffers: halo columns zeroed once, data region rewritten
        # per (seq, layer) via the shrink evacuation
        h_tiles = []
        for i in range(NH):
            t = wp.tile([2 * DP, HW_], BF16, tag=f"h{i}", name=f"h{i}")
            nc.gpsimd.memset(t[:, 0:HALO], 0.0)
            nc.gpsimd.memset(t[:, HALO + NP_ : HW_], 0.0)
            h_tiles.append(t)

        class Seq:
            def __init__(self, seq):
                self.seq = seq
                self.e = None
                self.o = None
                self.f = None

        hctr = [0]

        def stage_load(st):
            st.x = xp.tile([F, T], F32R, name="x_sb")
            nc.sync.dma_start(out=st.x[:], in_=xt_d[st.seq])

        def stage_unit0(st):
            st.e = ep.tile([DL, T], F32R, name="e_sb")
            for w in range(4):
                pe = eps.tile([DL, 512], F32, tag="pe", name="pe")
                nc.tensor.matmul(pe[:], we0_sb[:], st.x[:, w * 512 : (w + 1) * 512])
                nc.scalar.activation(
                    st.e[:, w * 512 : (w + 1) * 512],
                    pe[:],
                    AF.Relu,
                    bias=bias_sb[:, 0:1],
                    scale=1.0,
                )

        def expand(dst_sb, lcol, bias_col, o_prev):
            # dst[:, half*NP_ + t] = relu(We^T o_prev_half[:, t] + bias)
            for half in range(2):
                wcol = (lcol * 2 + half) * DL
                for w in range(2):
                    pe = eps.tile([DL, 512], F32, tag="pe", name="pe")
                    nc.tensor.matmul(
                        pe[:],
                        weil_sb[:, wcol : wcol + DL],
                        o_prev[:, w * 512 : (w + 1) * 512],
                    )
                    nc.scalar.activation(
                        dst_sb[:, half * NP_ + w * 512 : half * NP_ + (w + 1) * 512],
                        pe[:],
                        AF.Relu,
                        bias=bias_sb[:, bias_col : bias_col + 1],
                        scale=1.0,
                    )

        def stage_layer(st, l):
            if l > 0:
                e_new = ep.tile([DL, T], F32R, name="e_sb")
                expand(e_new, l - 1, l, st.o)
                st.e = e_new

            # ---- shrink l: weight-major, ev/od accumulate into one bank ----
            h_ps = [
                hps.tile([2 * DP, 512], F32, tag="hp", name=f"hps{w}")
                for w in range(2)
            ]
            for half in range(2):
                wcol = (l * 2 + half) * DL
                for w in range(2):
                    nc.tensor.matmul(
                        h_ps[w][:],
                        wsil_sb[:, wcol : wcol + DL],
                        st.e[:, half * NP_ + w * 512 : half * NP_ + (w + 1) * 512],
                        start=(half == 0),
                        stop=(half == 1),
                    )
            h_sb = h_tiles[hctr[0] % NH]
            hctr[0] += 1
            for w in range(2):
                nc.vector.tensor_copy(
                    h_sb[:, HALO + w * 512 : HALO + (w + 1) * 512], h_ps[w][:]
                )

            # ---- FSMN conv: 7 pair-shift passes, weight-major ----
            cp = [
                cps.tile([2 * DP, 512], F32, tag="cp", name=f"cp{w}")
                for w in range(2)
            ]
            for kk in range(NK):
                ck = (l * NK + kk) * 2 * DP
                for w in range(2):
                    nc.tensor.matmul(
                        cp[w][:],
                        wconv_sb[:, ck : ck + 2 * DP],
                        h_sb[:, w * 512 + kk : w * 512 + kk + 512],
                        start=(kk == 0),
                        stop=(kk == NK - 1),
                    )
            # ---- evacuate conv PSUM (+ residual for l>0) ----
            o_new = op_.tile([2 * DP, NP_], F32R, name="o_sb")
            for w in range(2):
                ws_ = slice(w * 512, (w + 1) * 512)
                if l == 0:
                    nc.vector.tensor_copy(o_new[:, ws_], cp[w][:])
                else:
                    nc.vector.tensor_tensor(o_new[:, ws_], cp[w][:], st.o[:, ws_], OP.add)
            st.o = o_new

        def stage_final(st):
            st.f = fp.tile([DL, T], BF16, name="f_sb")
            expand(st.f, L - 1, L, st.o)

        def stage_batch_out(b, f_tiles):
            pooled = pp.tile([DL, T], BF16, name="pooled")
            nc.vector.tensor_max(pooled[:], f_tiles[0][:], f_tiles[1][:])
            nc.vector.tensor_max(pooled[:], pooled[:], f_tiles[2][:])
            nc.vector.tensor_max(pooled[:], pooled[:], f_tiles[3][:])
            out_sb = osb.tile([S, T], F32, name="out_sb")
            for w in range(T // 512):
                pd = eps.tile([S, 512], F32, tag="pe", name="pd")
                nc.tensor.matmul(pd[:], wd_sb[:], pooled[:, w * 512 : (w + 1) * 512])
                nc.scalar.activation(
                    out_sb[:, w * 512 : (w + 1) * 512],
                    pd[:],
                    AF.Identity,
                    bias=bd_sb[:, 0:1],
                    scale=1.0,
                )
            nc.sync.dma_start(out=out_d[b], in_=out_sb[:])

        # ---- software-pipelined pairs of sequences ----
        f_by_batch = {b: [None] * C for b in range(BPC)}
        for pair in range(SEQ // 2):
            sA, sB = Seq(2 * pair), Seq(2 * pair + 1)
            for st in (sA, sB):
                stage_load(st)
            for st in (sA, sB):
                stage_unit0(st)
            for l in range(L):
                for st in (sA, sB):
                    stage_layer(st, l)
            for st in (sA, sB):
                stage_final(st)
                f_by_batch[st.seq // C][st.seq % C] = st.f
            # batch complete after its second pair
            b = sA.seq // C
            if sB.seq % C == C - 1:
                stage_batch_out(b, f_by_batch[b])
                f_by_batch[b] = [None] * C

    nc.compile()
    return nc


_NC = None


def get_nc():
    global _NC
    if _NC is None:
        _NC = build_nc()
    return _NC


def prep_in_maps(x, We0, be0, Ws0, wl0, wr0, We, be, Ws, wl, wr, We2, be2, Wd, bd):
    import ml_dtypes

    # x [B, T, C, F] -> [B, C, F, T] with even|odd time blocks
    xb = np.ascontiguousarray(x.transpose(0, 2, 3, 1), dtype=np.float32)
    xt = np.concatenate([xb[..., 0::2], xb[..., 1::2]], axis=-1)  # [B, C, F, T]

    ev = 2 * np.arange(DP)
    od = ev + 1

    # expand weights, zero-interleaved rows: o interleaved -> e
    we_list = [We[0], We[1], We[2], We[3], We2]
    weil = np.zeros((L, 2, DL, DL), np.float32)
    for l in range(L):
        weil[l, 0][ev, :] = we_list[l]
        weil[l, 1][od, :] = we_list[l]

    # shrink weights, zero-interleaved columns: e -> h interleaved
    ws_list = [Ws0, Ws[0], Ws[1], Ws[2], Ws[3]]
    wsil = np.zeros((L, 2, DL, DL), np.float32)
    for l in range(L):
        wsil[l, 0][:, ev] = ws_list[l]
        wsil[l, 1][:, od] = ws_list[l]

    biases = np.stack([be0, be[0], be[1], be[2], be[3], be2], axis=1).astype(
        np.float32
    )  # [128, 6]

    wl_full = np.concatenate([wl0[None], wl], axis=0)  # [L, 10, 64]
    wr_full = np.concatenate([wr0[None], wr], axis=0)  # [L, 1, 64]
    taps64 = np.concatenate([wl_full, wr_full], axis=1).copy()  # [L, 11, 64], j = d+9
    taps64[:, LO - 1, :] += 1.0  # conv identity term (o = h + left + right)

    # pair-shift conv weights Wc[l, kk][p_in, p_out] in interleaved layout,
    # k = kk - 5: [2c,2c]=t_{2k}, [2c+1,2c+1]=t_{2k}, [2c+1,2c]=t_{2k+1},
    # [2c,2c+1]=t_{2k-1}
    Wc = np.zeros((L, NK, 2 * DP, 2 * DP), np.float32)
    for l in range(L):
        for kk in range(NK):
            k = kk - 5
            d = 2 * k
            if -9 <= d <= 1:
                Wc[l, kk][ev, ev] = taps64[l, d + 9]
                Wc[l, kk][od, od] = taps64[l, d + 9]
            d = 2 * k + 1
            if -9 <= d <= 1:
                Wc[l, kk][od, ev] = taps64[l, d + 9]
            d = 2 * k - 1
            if -9 <= d <= 1:
                Wc[l, kk][ev, od] = taps64[l, d + 9]
    wconv = np.ascontiguousarray(
        Wc.transpose(2, 0, 1, 3).reshape(2 * DP, L * NK * 2 * DP)
    ).astype(ml_dtypes.bfloat16)

    shared = dict(
        we0=np.ascontiguousarray(We0, dtype=np.float32),
        weil=weil,
        wsil=wsil,
        wconv=wconv,
        wd=np.ascontiguousarray(Wd, dtype=np.float32).astype(ml_dtypes.bfloat16),
        biases=np.ascontiguousarray(biases),
        bd=np.ascontiguousarray(bd.reshape(S, 1), dtype=np.float32),
    )
    in_maps = []
    for k in range(NCORES):
        xs = xt[k * BPC : (k + 1) * BPC].reshape(SEQ, F, T)
        m = dict(shared)
        m["xt"] = np.ascontiguousarray(xs)
        in_maps.append(m)
    return in_maps


def postprocess(results):
    full = np.concatenate([r["out"] for r in results], axis=0)  # [B, S, T] ev|od
    res = np.empty((B, T, S), np.float32)
    res[:, 0::2, :] = full[:, :, :NP_].transpose(0, 2, 1)
    res[:, 1::2, :] = full[:, :, NP_:].transpose(0, 2, 1)
    return res


def kernel(**inputs):
    nc = get_nc()
    in_maps = prep_in_maps(**inputs)
    res = run_bass_kernel_spmd(nc, in_maps, core_ids=list(range(NCORES)))
    return postprocess(res.results)


# revision 8
# speedup vs baseline: 2.2490x; 1.0258x over previous
"""Trainium2 Bass kernel for nn_FSMNSeleNetV3 (FSMN stack + channel maxpool + decoder).

Self-contained: hardcodes all shapes from the problem spec and only imports
numpy + the concourse stack from /opt/trn_rl_repo.

Sharding: pure data parallel over batch. Each of the 8 cores processes 4
batches x 4 channels = 16 independent sequences of T=2048 tokens.

Layout: activations use an even/odd time-split layout, all in bf16 (fp32
PSUM accumulation). The 128-dim expand stream e is [128 feat, T] with
columns 0:1024 = even times, 1024:2048 = odd times. The 64-dim FSMN h
stream uses an interleaved pair layout (partition 2c = channel c even
times, 2c+1 = odd times); the conv output o uses a blocked pair layout
(partitions 0:63 = even, 64:127 = odd) so the expand can run as K=64
row-tiled concurrent matmul pairs (duplicated weight halves at PE rows 0
and 64 share the rhs stream).

FSMN conv: in pair layout the 11-tap depthwise conv collapses to 7
pair-shift matmuls per 512-column window. Each pass is a full 128x128 bf16
matmul whose weight is a banded matrix of per-channel 2x2 blocks with
interleaved input rows and blocked output columns (built on the host); all
taps accumulate in fp32 PSUM. The conv identity term (o = h + left +
right) is folded into the k=0 tap weights; the layer residual is folded
into the PSUM evacuation as a DVE tensor_tensor add.

The interleaved h layout keeps every matmul destination at PSUM base
partition 0 (the ISA rejects dst base 64): shrink even/odd are two M=128
matmuls with zero-interleaved weight columns accumulating into the same
bank.

Scheduling: sequences are emitted in software-pipelined pairs (stage-
interleaved) so the PE always has an independent matmul stream to fill
cross-engine latency gaps (keeps the PE HAM clock-gate warm). Matmuls that
share a stationary operand are emitted back-to-back; PSUM evacuations are
single wide-FD instructions. bf16 weights enable fast weight load (FWL).
"""

import sys

sys.path.insert(0, "/opt/trn_rl_repo")
from contextlib import ExitStack

import numpy as np

import concourse.bass as bass  # noqa: F401
import concourse.mybir as mybir
import concourse.tile as tile
from concourse import bacc
from concourse.bass_utils import run_bass_kernel_spmd

F32 = mybir.dt.float32
F32R = mybir.dt.float32r
BF16 = mybir.dt.bfloat16
AF = mybir.ActivationFunctionType
OP = mybir.AluOpType

NCORES = 8
B, T, C, F = 32, 2048, 4, 120
DL, DP, L, LO, RO, S = 128, 64, 5, 10, 1, 5
BPC = B // NCORES  # batches per core
SEQ = BPC * C  # sequences per core
NP_ = T // 2  # pair columns per sequence (1024)
HALO = 5  # left pair halo (k down to -5)
HW_ = HALO + NP_ + 1  # h buffer width: 1030
NK = 7  # pair-shift passes, k = kk - 5 in [-5 .. +1]
NH = 6  # static h buffers


def build_nc():
    nc = bacc.Bacc("TRN2", target_bir_lowering=False, debug=False, num_devices=NCORES)

    xt_d = nc.dram_tensor("xt", [SEQ, F, T], BF16, kind="ExternalInput")
    we0_d = nc.dram_tensor("we0", [F, DL], BF16, kind="ExternalInput")
    wedup_d = nc.dram_tensor("wedup", [L, 2 * DP, DL], BF16, kind="ExternalInput")
    wsil_d = nc.dram_tensor("wsil", [L, 2, DL, DL], BF16, kind="ExternalInput")
    wconv_d = nc.dram_tensor("wconv", [2 * DP, L * NK * 2 * DP], BF16, kind="ExternalInput")
    wd_d = nc.dram_tensor("wd", [DL, S], BF16, kind="ExternalInput")
    biases_d = nc.dram_tensor("biases", [DL, L + 1], F32, kind="ExternalInput")
    bd_d = nc.dram_tensor("bd", [S, 1], F32, kind="ExternalInput")
    out_d = nc.dram_tensor("out", [BPC, S, T], F32, kind="ExternalOutput")

    with tile.TileContext(nc) as tc, ExitStack() as ctx:
        wp = ctx.enter_context(tc.tile_pool(name="weights", bufs=1))
        xp = ctx.enter_context(tc.tile_pool(name="x", bufs=4))
        ep = ctx.enter_context(tc.tile_pool(name="e", bufs=4))
        op_ = ctx.enter_context(tc.tile_pool(name="o", bufs=4))
        fp = ctx.enter_context(tc.tile_pool(name="f", bufs=6))
        pp = ctx.enter_context(tc.tile_pool(name="pooled", bufs=2))
        osb = ctx.enter_context(tc.tile_pool(name="osb", bufs=2))
        eps = ctx.enter_context(tc.tile_pool(name="eps", bufs=3, space="PSUM"))
        hps = ctx.enter_context(tc.tile_pool(name="hps", bufs=2, space="PSUM"))
        cps = ctx.enter_context(tc.tile_pool(name="cps", bufs=3, space="PSUM"))

        # --- weights / constants (loaded once) ---
        we0_sb = wp.tile([F, DL], BF16)
        nc.sync.dma_start(out=we0_sb[:], in_=we0_d[:])
        wedup_sb = wp.tile([2 * DP, L * DL], BF16)
        wsil_sb = wp.tile([DL, L * 2 * DL], BF16)
        for l in range(L):
            nc.sync.dma_start(out=wedup_sb[:, l * DL : (l + 1) * DL], in_=wedup_d[l])
            for half in range(2):
                col = (l * 2 + half) * DL
                nc.sync.dma_start(out=wsil_sb[:, col : col + DL], in_=wsil_d[l, half])
        wconv_sb = wp.tile([2 * DP, L * NK * 2 * DP], BF16)
        nc.sync.dma_start(out=wconv_sb[:], in_=wconv_d[:])
        wd_sb = wp.tile([DL, S], BF16)
        nc.sync.dma_start(out=wd_sb[:], in_=wd_d[:])
        bias_sb = wp.tile([DL, L + 1], F32)
        nc.sync.dma_start(out=bias_sb[:], in_=biases_d[:])
        bd_sb = wp.tile([S, 1], F32)
        nc.sync.dma_start(out=bd_sb[:], in_=bd_d[:])

        # static h buffers: halo columns zeroed once, data region rewritten
        # per (seq, layer) via the shrink evacuation
        h_tiles = []
        for i in range(NH):
            t = wp.tile([2 * DP, HW_], BF16, tag=f"h{i}", name=f"h{i}")
            nc.gpsimd.memset(t[:, 0:HALO], 0.0)
            nc.gpsimd.memset(t[:, HALO + NP_ : HW_], 0.0)
            h_tiles.append(t)

        class Seq:
            def __init__(self, seq):
                self.seq = seq
                self.e = None
                self.o = None
                self.f = None

        hctr = [0]

        def stage_load(st):
            st.x = xp.tile([F, T], BF16, name="x_sb")
            nc.sync.dma_start(out=st.x[:], in_=xt_d[st.seq])

        def stage_unit0(st):
            st.e = ep.tile([DL, T], BF16, name="e_sb")
            for w in range(4):
                pe = eps.tile([DL, 512], F32, tag="pe", name="pe")
                nc.tensor.matmul(pe[:], we0_sb[:], st.x[:, w * 512 : (w + 1) * 512])
                nc.scalar.activation(
                    st.e[:, w * 512 : (w + 1) * 512],
                    pe[:],
                    AF.Relu,
                    bias=bias_sb[:, 0:1],
                    scale=1.0,
                )

        def expand(dst_sb, lcol, bias_col, o_prev):
            # o_prev blocked: rows 0:63 = even half, 64:127 = odd half.
            # K=64 row-tiled pairs (weights duplicated at rows 0 and 64)
            # stream concurrently and share the rhs columns.
            for w in range(2):
                ws_ = slice(w * 512, (w + 1) * 512)
                pes = []
                for half in range(2):
                    q = half * DP
                    pe = eps.tile([DL, 512], F32, tag="pe", name="pe")
                    nc.tensor.matmul(
                        pe[:],
                        wedup_sb[q : q + DP, lcol * DL : (lcol + 1) * DL],
                        o_prev[q : q + DP, ws_],
                        tile_position=(q, 0),
                    )
                    pes.append(pe)
                for half in range(2):
                    nc.scalar.activation(
                        dst_sb[:, half * NP_ + w * 512 : half * NP_ + (w + 1) * 512],
                        pes[half][:],
                        AF.Relu,
                        bias=bias_sb[:, bias_col : bias_col + 1],
                        scale=1.0,
                    )

        def stage_layer(st, l):
            if l > 0:
                e_new = ep.tile([DL, T], BF16, name="e_sb")
                expand(e_new, l - 1, l, st.o)
                st.e = e_new

            # ---- shrink l: weight-major, ev/od accumulate into one bank ----
            h_ps = [
                hps.tile([2 * DP, 512], F32, tag="hp", name=f"hps{w}")
                for w in range(2)
            ]
            for half in range(2):
                wcol = (l * 2 + half) * DL
                for w in range(2):
                    nc.tensor.matmul(
                        h_ps[w][:],
                        wsil_sb[:, wcol : wcol + DL],
                        st.e[:, half * NP_ + w * 512 : half * NP_ + (w + 1) * 512],
                        start=(half == 0),
                        stop=(half == 1),
                    )
            h_sb = h_tiles[hctr[0] % NH]
            hctr[0] += 1
            for w in range(2):
                nc.vector.tensor_copy(
                    h_sb[:, HALO + w * 512 : HALO + (w + 1) * 512], h_ps[w][:]
                )

            # ---- FSMN conv: 7 pair-shift passes, weight-major ----
            cp = [
                cps.tile([2 * DP, 512], F32, tag="cp", name=f"cp{w}")
                for w in range(2)
            ]
            for kk in range(NK):
                ck = (l * NK + kk) * 2 * DP
                for w in range(2):
                    nc.tensor.matmul(
                        cp[w][:],
                        wconv_sb[:, ck : ck + 2 * DP],
                        h_sb[:, w * 512 + kk : w * 512 + kk + 512],
                        start=(kk == 0),
                        stop=(kk == NK - 1),
                    )
            # ---- evacuate conv PSUM (+ residual for l>0), o blocked ----
            o_new = op_.tile([2 * DP, NP_], BF16, name="o_sb")
            for w in range(2):
                ws_ = slice(w * 512, (w + 1) * 512)
                if l == 0:
                    nc.vector.tensor_copy(o_new[:, ws_], cp[w][:])
                else:
                    nc.vector.tensor_tensor(o_new[:, ws_], cp[w][:], st.o[:, ws_], OP.add)
            st.o = o_new

        def stage_final(st):
            st.f = fp.tile([DL, T], BF16, name="f_sb")
            expand(st.f, L - 1, L, st.o)

        def stage_batch_out(b, f_tiles):
            pooled = pp.tile([DL, T], BF16, name="pooled")
            nc.vector.tensor_max(pooled[:], f_tiles[0][:], f_tiles[1][:])
            nc.vector.tensor_max(pooled[:], pooled[:], f_tiles[2][:])
            nc.vector.tensor_max(pooled[:], pooled[:], f_tiles[3][:])
            out_sb = osb.tile([S, T], F32, name="out_sb")
            for w in range(T // 512):
                pd = eps.tile([S, 512], F32, tag="pe", name="pd")
                nc.tensor.matmul(pd[:], wd_sb[:], pooled[:, w * 512 : (w + 1) * 512])
                nc.scalar.activation(
                    out_sb[:, w * 512 : (w + 1) * 512],
                    pd[:],
                    AF.Identity,
                    bias=bd_sb[:, 0:1],
                    scale=1.0,
                )
            nc.sync.dma_start(out=out_d[b], in_=out_sb[:])

        # ---- software-pipelined pairs of sequences ----
        f_by_batch = {b: [None] * C for b in range(BPC)}
        for pair in range(SEQ // 2):
            sA, sB = Seq(2 * pair), Seq(2 * pair + 1)
            for st in (sA, sB):
                stage_load(st)
            for st in (sA, sB):
                stage_unit0(st)
            for l in range(L):
                for st in (sA, sB):
                    stage_layer(st, l)
            for st in (sA, sB):
                stage_final(st)
                f_by_batch[st.seq // C][st.seq % C] = st.f
            b = sA.seq // C
            if sB.seq % C == C - 1:
                stage_batch_out(b, f_by_batch[b])
                f_by_batch[b] = [None] * C

    nc.compile()
    return nc


_NC = None


def get_nc():
    global _NC
    if _NC is None:
        _NC = build_nc()
    return _NC


def prep_in_maps(x, We0, be0, Ws0, wl0, wr0, We, be, Ws, wl, wr, We2, be2, Wd, bd):
    import ml_dtypes

    bf16 = ml_dtypes.bfloat16

    # x [B, T, C, F] -> [B, C, F, T] with even|odd time blocks
    xb = np.ascontiguousarray(x.transpose(0, 2, 3, 1), dtype=np.float32)
    xt = np.concatenate([xb[..., 0::2], xb[..., 1::2]], axis=-1).astype(bf16)

    ev = 2 * np.arange(DP)
    od = ev + 1

    # expand weights: o blocked -> duplicated halves for row-tiled pairs
    we_list = [We[0], We[1], We[2], We[3], We2]
    wedup = np.stack([np.concatenate([w, w], axis=0) for w in we_list]).astype(bf16)

    # shrink weights, zero-interleaved columns: e -> h interleaved
    ws_list = [Ws0, Ws[0], Ws[1], Ws[2], Ws[3]]
    wsil = np.zeros((L, 2, DL, DL), np.float32)
    for l in range(L):
        wsil[l, 0][:, ev] = ws_list[l]
        wsil[l, 1][:, od] = ws_list[l]

    biases = np.stack([be0, be[0], be[1], be[2], be[3], be2], axis=1).astype(
        np.float32
    )  # [128, 6]

    wl_full = np.concatenate([wl0[None], wl], axis=0)  # [L, 10, 64]
    wr_full = np.concatenate([wr0[None], wr], axis=0)  # [L, 1, 64]
    taps64 = np.concatenate([wl_full, wr_full], axis=1).copy()  # [L, 11, 64], j = d+9
    taps64[:, LO - 1, :] += 1.0  # conv identity term (o = h + left + right)

    # pair-shift conv weights Wc[l, kk][p_in, p_out]: input rows interleaved
    # (2c = ev, 2c+1 = od), output cols blocked (c = ev, c+64 = od).
    # k = kk - 5: ev_out<-ev_in t_{2k}, ev_out<-od_in t_{2k+1},
    # od_out<-ev_in t_{2k-1}, od_out<-od_in t_{2k}
    Wc = np.zeros((L, NK, 2 * DP, 2 * DP), np.float32)
    cc = np.arange(DP)
    for l in range(L):
        for kk in range(NK):
            k = kk - 5
            d = 2 * k
            if -9 <= d <= 1:
                Wc[l, kk][ev, cc] = taps64[l, d + 9]
                Wc[l, kk][od, cc + DP] = taps64[l, d + 9]
            d = 2 * k + 1
            if -9 <= d <= 1:
                Wc[l, kk][od, cc] = taps64[l, d + 9]
            d = 2 * k - 1
            if -9 <= d <= 1:
                Wc[l, kk][ev, cc + DP] = taps64[l, d + 9]
    wconv = np.ascontiguousarray(
        Wc.transpose(2, 0, 1, 3).reshape(2 * DP, L * NK * 2 * DP)
    ).astype(bf16)

    shared = dict(
        we0=np.ascontiguousarray(We0).astype(bf16),
        wedup=wedup,
        wsil=wsil.astype(bf16),
        wconv=wconv,
        wd=np.ascontiguousarray(Wd).astype(bf16),
        biases=np.ascontiguousarray(biases),
        bd=np.ascontiguousarray(bd.reshape(S, 1), dtype=np.float32),
    )
    in_maps = []
    for k in range(NCORES):
        xs = xt[k * BPC : (k + 1) * BPC].reshape(SEQ, F, T)
        m = dict(shared)
        m["xt"] = np.ascontiguousarray(xs)
        in_maps.append(m)
    return in_maps


def postprocess(results):
    full = np.concatenate([r["out"] for r in results], axis=0)  # [B, S, T] ev|od
    res = np.empty((B, T, S), np.float32)
    res[:, 0::2, :] = full[:, :, :NP_].transpose(0, 2, 1)
    res[:, 1::2, :] = full[:, :, NP_:].transpose(0, 2, 1)
    return res


def kernel(**inputs):
    nc = get_nc()
    in_maps = prep_in_maps(**inputs)
    res = run_bass_kernel_spmd(nc, in_maps, core_ids=list(range(NCORES)))
    return postprocess(res.results)


# revision 15
# speedup vs baseline: 2.5370x; 1.1281x over previous
"""Trainium2 Bass kernel for nn_FSMNSeleNetV3 (FSMN stack + channel maxpool + decoder).

Self-contained: hardcodes all shapes from the problem spec and only imports
numpy + the concourse stack from /opt/trn_rl_repo.

Sharding: pure data parallel over batch. Each of the 8 cores processes 4
batches x 4 channels = 16 independent sequences of T=2048 tokens.

Layout: activations use an even/odd time-split layout, all in bf16 (fp32
PSUM accumulation). The 128-dim expand stream e is [128 feat, T] with
columns 0:1024 = even times, 1024:2048 = odd times. The 64-dim FSMN h
stream uses an interleaved pair layout (partition 2c = channel c even
times, 2c+1 = odd times); the conv output o uses a blocked pair layout
(partitions 0:63 = even, 64:127 = odd) so the expand can run as K=64
row-tiled concurrent matmul pairs (duplicated weight halves at PE rows 0
and 64 share the rhs stream).

FSMN conv: in pair layout the 11-tap depthwise conv collapses to 7
pair-shift matmuls per 512-column window. Each pass is a full 128x128 bf16
matmul whose weight is a banded matrix of per-channel 2x2 blocks with
interleaved input rows and blocked output columns (built on the host); all
taps accumulate in fp32 PSUM. The conv identity term (o = h + left +
right) is folded into the k=0 tap weights; the layer residual is folded
into the PSUM evacuation as a DVE tensor_tensor add.

The interleaved h layout keeps every matmul destination at PSUM base
partition 0 (the ISA rejects dst base 64): shrink even/odd are two M=128
matmuls with zero-interleaved weight columns accumulating into the same
bank.

Scheduling: sequences are emitted in software-pipelined pairs (stage-
interleaved) so the PE always has an independent matmul stream to fill
cross-engine latency gaps (keeps the PE HAM clock-gate warm). Matmuls that
share a stationary operand are emitted back-to-back; PSUM evacuations are
single wide-FD instructions. bf16 weights enable fast weight load (FWL).
"""

import sys

sys.path.insert(0, "/opt/trn_rl_repo")
from contextlib import ExitStack

import numpy as np

import concourse.bass as bass  # noqa: F401
import concourse.mybir as mybir
import concourse.tile as tile
from concourse import bacc
from concourse.bass_utils import run_bass_kernel_spmd

F32 = mybir.dt.float32
F32R = mybir.dt.float32r
BF16 = mybir.dt.bfloat16
AF = mybir.ActivationFunctionType
OP = mybir.AluOpType

NCORES = 8
B, T, C, F = 32, 2048, 4, 120
DL, DP, L, LO, RO, S = 128, 64, 5, 10, 1, 5
BPC = B // NCORES  # batches per core
SEQ = BPC * C  # sequences per core
NP_ = T // 2  # pair columns per sequence (1024)
HALO = 5  # left pair halo (k down to -5)
HW_ = HALO + NP_ + 1  # h buffer width: 1030
NK = 7  # pair-shift passes, k = kk - 5 in [-5 .. +1]
NH = 6  # static h buffers

# packed bf16 weight tensor column offsets
OFF_WE0 = 0
OFF_WEDUP = OFF_WE0 + DL
OFF_WSIL = OFF_WEDUP + L * DL
OFF_WCONV = OFF_WSIL + L * 2 * DL
OFF_WD = OFF_WCONV + L * NK * 2 * DP
WPK_COLS = OFF_WD + 8


def build_nc():
    nc = bacc.Bacc("TRN2", target_bir_lowering=False, debug=False, num_devices=NCORES)

    xt_d = nc.dram_tensor("xt", [SEQ, F, T], BF16, kind="ExternalInput")
    wpk_d = nc.dram_tensor("wpk", [DL, WPK_COLS], BF16, kind="ExternalInput")
    wpk32_d = nc.dram_tensor("wpk32", [DL, 8], F32, kind="ExternalInput")
    out_d = nc.dram_tensor("out", [BPC, S, T], F32, kind="ExternalOutput")

    with tile.TileContext(nc) as tc, ExitStack() as ctx:
        wp = ctx.enter_context(tc.tile_pool(name="weights", bufs=1))
        xp = ctx.enter_context(tc.tile_pool(name="x", bufs=4))
        ep = ctx.enter_context(tc.tile_pool(name="e", bufs=4))
        op_ = ctx.enter_context(tc.tile_pool(name="o", bufs=4))
        fp = ctx.enter_context(tc.tile_pool(name="f", bufs=6))
        pp = ctx.enter_context(tc.tile_pool(name="pooled", bufs=2))
        osb = ctx.enter_context(tc.tile_pool(name="osb", bufs=2))
        eps = ctx.enter_context(tc.tile_pool(name="eps", bufs=4, space="PSUM"))
        hps = ctx.enter_context(tc.tile_pool(name="hps", bufs=2, space="PSUM"))
        cps = ctx.enter_context(tc.tile_pool(name="cps", bufs=2, space="PSUM"))

        # --- weights / constants (2 packed DMAs) ---
        wpk32_sb = wp.tile([DL, 8], F32)
        nc.sync.dma_start(out=wpk32_sb[:], in_=wpk32_d[:])
        wpk_sb = wp.tile([DL, WPK_COLS], BF16)
        nc.sync.dma_start(out=wpk_sb[:], in_=wpk_d[:])

        we0_sb = wpk_sb[0:F, OFF_WE0 : OFF_WE0 + DL]

        def wedup_at(l, q):
            c = OFF_WEDUP + l * DL
            return wpk_sb[q : q + DP, c : c + DL]

        def wsil_at(l, half):
            c = OFF_WSIL + (l * 2 + half) * DL
            return wpk_sb[:, c : c + DL]

        def wconv_at(l, kk):
            c = OFF_WCONV + (l * NK + kk) * 2 * DP
            return wpk_sb[:, c : c + 2 * DP]

        wd_sb = wpk_sb[:, OFF_WD : OFF_WD + S]
        bias_sb = wpk32_sb
        bd_sb = wpk32_sb[0:S, 6:7]

        # static h buffers: halo columns zeroed once, data region rewritten
        # per (seq, layer) via the shrink evacuation
        h_tiles = []
        for i in range(NH):
            t = wp.tile([2 * DP, HW_], BF16, tag=f"h{i}", name=f"h{i}")
            nc.gpsimd.memset(t[:, 0:HALO], 0.0)
            nc.gpsimd.memset(t[:, HALO + NP_ : HW_], 0.0)
            h_tiles.append(t)

        class Seq:
            def __init__(self, seq):
                self.seq = seq
                self.e = None
                self.o = None
                self.f = None

        hctr = [0]

        def stage_load(st):
            st.x = xp.tile([F, T], BF16, name="x_sb")
            nc.sync.dma_start(out=st.x[:], in_=xt_d[st.seq])

        def stage_unit0(st):
            st.e = ep.tile([DL, T], BF16, name="e_sb")
            for w in range(4):
                pe = eps.tile([DL, 512], F32, tag="pe", name="pe")
                nc.tensor.matmul(pe[:], we0_sb, st.x[:, w * 512 : (w + 1) * 512])
                nc.scalar.activation(
                    st.e[:, w * 512 : (w + 1) * 512],
                    pe[:],
                    AF.Relu,
                    bias=bias_sb[:, 0:1],
                    scale=1.0,
                )

        def expand(dst_sb, lcol, bias_col, o_prev):
            # o_prev blocked: rows 0:63 = even half, 64:127 = odd half.
            # K=64 row-tiled pairs (weights duplicated at rows 0 and 64)
            # stream concurrently and share the rhs columns.
            for w in range(2):
                ws_ = slice(w * 512, (w + 1) * 512)
                pes = []
                for half in range(2):
                    q = half * DP
                    pe = eps.tile([DL, 512], F32, tag="pe", name="pe")
                    nc.tensor.matmul(
                        pe[:],
                        wedup_at(lcol, q),
                        o_prev[q : q + DP, ws_],
                        tile_position=(q, 0),
                    )
                    pes.append(pe)
                for half in range(2):
                    nc.scalar.activation(
                        dst_sb[:, half * NP_ + w * 512 : half * NP_ + (w + 1) * 512],
                        pes[half][:],
                        AF.Relu,
                        bias=bias_sb[:, bias_col : bias_col + 1],
                        scale=1.0,
                    )

        def stage_layer(st, l):
            if l > 0:
                e_new = ep.tile([DL, T], BF16, name="e_sb")
                expand(e_new, l - 1, l, st.o)
                st.e = e_new

            # ---- shrink l: weight-major, ev/od accumulate into one bank ----
            h_ps = [
                hps.tile([2 * DP, 512], F32, tag="hp", name=f"hps{w}")
                for w in range(2)
            ]
            for half in range(2):
                for w in range(2):
                    nc.tensor.matmul(
                        h_ps[w][:],
                        wsil_at(l, half),
                        st.e[:, half * NP_ + w * 512 : half * NP_ + (w + 1) * 512],
                        start=(half == 0),
                        stop=(half == 1),
                    )
            h_sb = h_tiles[hctr[0] % NH]
            hctr[0] += 1
            for w in range(2):
                nc.vector.tensor_copy(
                    h_sb[:, HALO + w * 512 : HALO + (w + 1) * 512], h_ps[w][:]
                )

            # ---- FSMN conv: 7 pair-shift passes, weight-major ----
            cp = [
                cps.tile([2 * DP, 512], F32, tag="cp", name=f"cp{w}")
                for w in range(2)
            ]
            for kk in range(NK):
                for w in range(2):
                    nc.tensor.matmul(
                        cp[w][:],
                        wconv_at(l, kk),
                        h_sb[:, w * 512 + kk : w * 512 + kk + 512],
                        start=(kk == 0),
                        stop=(kk == NK - 1),
                    )
            # ---- evacuate conv PSUM (+ residual for l>0), o blocked ----
            o_new = op_.tile([2 * DP, NP_], BF16, name="o_sb")
            for w in range(2):
                ws_ = slice(w * 512, (w + 1) * 512)
                if l == 0:
                    nc.vector.tensor_copy(o_new[:, ws_], cp[w][:])
                else:
                    nc.vector.tensor_tensor(o_new[:, ws_], cp[w][:], st.o[:, ws_], OP.add)
            st.o = o_new

        def stage_final(st):
            st.f = fp.tile([DL, T], BF16, name="f_sb")
            expand(st.f, L - 1, L, st.o)

        def stage_batch_out(b, f_tiles):
            pooled = pp.tile([DL, T], BF16, name="pooled")
            nc.vector.tensor_max(pooled[:], f_tiles[0][:], f_tiles[1][:])
            nc.vector.tensor_max(pooled[:], pooled[:], f_tiles[2][:])
            nc.vector.tensor_max(pooled[:], pooled[:], f_tiles[3][:])
            out_sb = osb.tile([S, T], F32, name="out_sb")
            for w in range(T // 512):
                pd = eps.tile([S, 512], F32, tag="pe", name="pd")
                nc.tensor.matmul(pd[:], wd_sb, pooled[:, w * 512 : (w + 1) * 512])
                nc.scalar.activation(
                    out_sb[:, w * 512 : (w + 1) * 512],
                    pd[:],
                    AF.Identity,
                    bias=bd_sb,
                    scale=1.0,
                )
            nc.sync.dma_start(out=out_d[b], in_=out_sb[:])

        # ---- software-pipelined pairs of sequences; the batch output
        # (pool + decode) is deferred into the next pair's layer stream so
        # the PE never waits on it ----
        f_by_batch = {b: [None] * C for b in range(BPC)}
        pending_out = [None]
        for pair in range(SEQ // 2):
            sA, sB = Seq(2 * pair), Seq(2 * pair + 1)
            for st in (sA, sB):
                stage_load(st)
            for st in (sA, sB):
                stage_unit0(st)
            for l in range(L):
                for st in (sA, sB):
                    stage_layer(st, l)
                if l == 0 and pending_out[0] is not None:
                    b_out, fs = pending_out[0]
                    stage_batch_out(b_out, fs)
                    pending_out[0] = None
            for st in (sA, sB):
                stage_final(st)
                f_by_batch[st.seq // C][st.seq % C] = st.f
            b = sA.seq // C
            if sB.seq % C == C - 1:
                pending_out[0] = (b, f_by_batch[b])
                f_by_batch[b] = [None] * C
        if pending_out[0] is not None:
            b_out, fs = pending_out[0]
            stage_batch_out(b_out, fs)

    nc.compile()
    return nc


_NC = None


def get_nc():
    global _NC
    if _NC is None:
        _NC = build_nc()
    return _NC


def prep_in_maps(x, We0, be0, Ws0, wl0, wr0, We, be, Ws, wl, wr, We2, be2, Wd, bd):
    import ml_dtypes

    bf16 = ml_dtypes.bfloat16

    # x [B, T, C, F] -> [B, C, F, T] with even|odd time blocks
    xb = np.ascontiguousarray(x.transpose(0, 2, 3, 1), dtype=np.float32)
    xt = np.concatenate([xb[..., 0::2], xb[..., 1::2]], axis=-1).astype(bf16)

    ev = 2 * np.arange(DP)
    od = ev + 1

    # expand weights: o blocked -> duplicated halves for row-tiled pairs
    we_list = [We[0], We[1], We[2], We[3], We2]
    wedup = np.stack([np.concatenate([w, w], axis=0) for w in we_list]).astype(bf16)

    # shrink weights, zero-interleaved columns: e -> h interleaved
    ws_list = [Ws0, Ws[0], Ws[1], Ws[2], Ws[3]]
    wsil = np.zeros((L, 2, DL, DL), np.float32)
    for l in range(L):
        wsil[l, 0][:, ev] = ws_list[l]
        wsil[l, 1][:, od] = ws_list[l]

    biases = np.stack([be0, be[0], be[1], be[2], be[3], be2], axis=1).astype(
        np.float32
    )  # [128, 6]

    wl_full = np.concatenate([wl0[None], wl], axis=0)  # [L, 10, 64]
    wr_full = np.concatenate([wr0[None], wr], axis=0)  # [L, 1, 64]
    taps64 = np.concatenate([wl_full, wr_full], axis=1).copy()  # [L, 11, 64], j = d+9
    taps64[:, LO - 1, :] += 1.0  # conv identity term (o = h + left + right)

    # pair-shift conv weights Wc[l, kk][p_in, p_out]: input rows interleaved
    # (2c = ev, 2c+1 = od), output cols blocked (c = ev, c+64 = od).
    # k = kk - 5: ev_out<-ev_in t_{2k}, ev_out<-od_in t_{2k+1},
    # od_out<-ev_in t_{2k-1}, od_out<-od_in t_{2k}
    Wc = np.zeros((L, NK, 2 * DP, 2 * DP), np.float32)
    cc = np.arange(DP)
    for l in range(L):
        for kk in range(NK):
            k = kk - 5
            d = 2 * k
            if -9 <= d <= 1:
                Wc[l, kk][ev, cc] = taps64[l, d + 9]
                Wc[l, kk][od, cc + DP] = taps64[l, d + 9]
            d = 2 * k + 1
            if -9 <= d <= 1:
                Wc[l, kk][od, cc] = taps64[l, d + 9]
            d = 2 * k - 1
            if -9 <= d <= 1:
                Wc[l, kk][ev, cc + DP] = taps64[l, d + 9]
    wconv = np.ascontiguousarray(
        Wc.transpose(2, 0, 1, 3).reshape(2 * DP, L * NK * 2 * DP)
    ).astype(bf16)

    wpk = np.zeros((DL, WPK_COLS), bf16)
    wpk[0:F, OFF_WE0 : OFF_WE0 + DL] = We0.astype(bf16)
    wpk[:, OFF_WEDUP : OFF_WEDUP + L * DL] = wedup.transpose(1, 0, 2).reshape(
        DL, L * DL
    )
    wpk[:, OFF_WSIL : OFF_WSIL + 2 * L * DL] = (
        wsil.reshape(2 * L, DL, DL).transpose(1, 0, 2).reshape(DL, 2 * L * DL)
    ).astype(bf16)
    wpk[:, OFF_WCONV : OFF_WCONV + L * NK * 2 * DP] = wconv
    wpk[:, OFF_WD : OFF_WD + S] = Wd.astype(bf16)

    wpk32 = np.zeros((DL, 8), np.float32)
    wpk32[:, 0 : L + 1] = biases
    wpk32[0:S, 6] = bd

    shared = dict(wpk=np.ascontiguousarray(wpk), wpk32=wpk32)
    in_maps = []
    for k in range(NCORES):
        xs = xt[k * BPC : (k + 1) * BPC].reshape(SEQ, F, T)
        m = dict(shared)
        m["xt"] = np.ascontiguousarray(xs)
        in_maps.append(m)
    return in_maps


def postprocess(results):
    full = np.concatenate([r["out"] for r in results], axis=0)  # [B, S, T] ev|od
    res = np.empty((B, T, S), np.float32)
    res[:, 0::2, :] = full[:, :, :NP_].transpose(0, 2, 1)
    res[:, 1::2, :] = full[:, :, NP_:].transpose(0, 2, 1)
    return res


def kernel(**inputs):
    nc = get_nc()
    in_maps = prep_in_maps(**inputs)
    res = run_bass_kernel_spmd(nc, in_maps, core_ids=list(range(NCORES)))
    return postprocess(res.results)
